# revision 4
# baseline (speedup 1.0000x reference)
"""GAT message-passing (CentroidGATConv) Trainium2 Bass kernel, v2.

Strategy (8 NeuronCores, SPMD, no collectives):
  - Destination-node range sharding: core k owns dst nodes [k*npc, (k+1)*npc).
    dst is sorted, so each core's edges form contiguous per-block segments and
    every segment softmax/aggregate is core-local.
  - Phase 1 (GEMM, replicated, bf16): ftel = feat @ [W | W@AL | W@AR] giving
    per-node rows [ft(256) | el(4)] (written to two DRAM tables split at node
    32768 for int16 gather indices) + er_tab[n, 4].
  - Phase 2 (edges): per 2-block group, batch-gather all src rows with TWO
    dma_gather calls (lo/hi tables, int16 idx), build one-hot (oh) and its
    transpose (ohT, from host-provided mrelT) on DVE via iota-compare, route
    er to edges with tiny PE matmuls (lhsT=ohT, rhs=er_nodes), compute
    w = exp(leaky_relu(el+er)), aggregate with PE matmuls
    out[n] , den[n] += oh_s.T @ [w*ft | w], divide at node level.

kernel(**inputs) takes FULL inputs, shards internally, returns FULL output.
"""

import math
import os
from contextlib import ExitStack

import ml_dtypes
import numpy as np

import concourse.bacc as bacc
import concourse.bass as bass
import concourse.mybir as mybir
import concourse.tile as tile
from concourse.bass import IndirectOffsetOnAxis
from concourse.bass_utils import run_bass_kernel_spmd

F32 = mybir.dt.float32
BF16 = mybir.dt.bfloat16
FP8 = mybir.dt.float8e4
I32 = mybir.dt.int32
I16 = mybir.dt.int16
U8 = mybir.dt.uint8
FP8_ONE = 0x38            # bit pattern of 1.0 in fp8 e4m3
AF = mybir.ActivationFunctionType
OP = mybir.AluOpType

P = 128
N_CORES = 8
NEG_SLOPE = 0.2
SPLIT = 32768          # node-id split for int16 gather indices
ROW = 384              # padded ftel row (bf16): [ft 256 | el 4 | pad] = 768B
GROUP = 2              # dst-node blocks per gather group

LAST_RESULTS = None
_PROGRAM_CACHE = {}


class Cfg:
    def __init__(self, N, E, d_in, H, D, NSLO, NSHI):
        self.N = N
        self.E = E
        self.d_in = d_in
        self.H = H
        self.D = D
        self.hd = H * D
        self.npad = ((N + P - 1) // P) * P
        self.npc = (N + N_CORES - 1) // N_CORES
        self.n_loc_pad = ((self.npc + P - 1) // P) * P
        self.B = self.n_loc_pad // P
        self.NSLO = NSLO            # per-block lo subchunk counts [B]
        self.NSHI = NSHI
        self.KH = d_in // P
        assert d_in % P == 0
        assert self.hd + H <= ROW

        # group layout: blocks (2g, 2g+1); subchunk order per group:
        # [b0lo | b1lo | b0hi | b1hi]
        self.groups = []
        sub_off = 0
        lo_col = 0   # int16 idx cols consumed (slots/16)
        hi_col = 0
        for g0 in range(0, self.B, GROUP):
            blks = list(range(g0, min(g0 + GROUP, self.B)))
            nslo = [NSLO[b] for b in blks]
            nshi = [NSHI[b] for b in blks]
            ns_lo_g = sum(nslo)
            ns_hi_g = sum(nshi)
            ns_g = ns_lo_g + ns_hi_g
            # group-local subchunk ranges per block: (lo_start, lo_n, hi_start, hi_n)
            sec = []
            lo_s = 0
            hi_s = ns_lo_g
            for i in range(len(blks)):
                sec.append((lo_s, nslo[i], hi_s, nshi[i]))
                lo_s += nslo[i]
                hi_s += nshi[i]
            self.groups.append({
                "blks": blks, "sec": sec, "ns_lo": ns_lo_g, "ns_hi": ns_hi_g,
                "ns": ns_g, "sub_off": sub_off,
                "lo_col": lo_col, "hi_col": hi_col,
            })
            sub_off += ns_g
            lo_col += ns_lo_g * 8   # 128 slots/subchunk / 16 = 8 cols
            hi_col += ns_hi_g * 8
        self.S = sub_off
        self.L_LO = lo_col
        self.L_HI = hi_col
        self.NSG_MAX = max(g["ns"] for g in self.groups)

    def key(self):
        return (self.N, self.E, self.d_in, self.H, self.D,
                tuple(self.NSLO), tuple(self.NSHI))


def host_prep(feat, src, dst, W, attn_l, attn_r):
    feat = np.asarray(feat, dtype=np.float32)
    src = np.asarray(src).astype(np.int64)
    dst = np.asarray(dst).astype(np.int64)
    W = np.asarray(W, dtype=np.float32)
    attn_l = np.asarray(attn_l, dtype=np.float32)
    attn_r = np.asarray(attn_r, dtype=np.float32)

    N, d_in = feat.shape
    H, D = attn_l.shape
    E = src.shape[0]
    hd = H * D

    AL = np.zeros((hd, H), np.float32)
    AR = np.zeros((hd, H), np.float32)
    for h in range(H):
        AL[h * D:(h + 1) * D, h] = attn_l[h]
        AR[h * D:(h + 1) * D, h] = attn_r[h]
    # cols: [ft(256) | el(4) | er(4)]
    wcomb = np.concatenate([W, W @ AL, W @ AR], axis=1).astype(ml_dtypes.bfloat16)

    npc = (N + N_CORES - 1) // N_CORES
    n_loc_pad = ((npc + P - 1) // P) * P
    B = n_loc_pad // P

    core = np.minimum(dst // npc, N_CORES - 1)
    loc = dst - core * npc
    blk = loc // P
    key = core * B + blk
    hi_f = (src >= SPLIT).astype(np.int64)
    n_all = np.bincount(key, minlength=N_CORES * B).reshape(N_CORES, B)
    n_lo = np.bincount(key[hi_f == 0], minlength=N_CORES * B).reshape(N_CORES, B)
    n_hi = n_all - n_lo
    NSLO = [int(x) for x in np.ceil(n_lo.max(axis=0) / P).astype(np.int64)]
    NSHI = [int(x) for x in np.ceil(n_hi.max(axis=0) / P).astype(np.int64)]

    cfg = Cfg(N, E, d_in, H, D, NSLO, NSHI)

    # per-edge rank within (core, block, section)
    key2 = key * 2 + hi_f
    order = np.argsort(key2, kind="stable")
    counts2 = np.bincount(key2, minlength=N_CORES * B * 2)
    starts2 = np.zeros_like(counts2)
    starts2[1:] = np.cumsum(counts2)[:-1]
    rank = np.empty(E, np.int64)
    rank[order] = np.arange(E) - starts2[key2[order]]

    # static per (block, section) bases
    lo_base_j = np.zeros(B, np.int64)    # j base within the group's LO call
    hi_base_j = np.zeros(B, np.int64)
    lo_call_col = np.zeros(B, np.int64)  # idx col offset of the block's group call
    hi_call_col = np.zeros(B, np.int64)
    sub_lo = np.zeros(B, np.int64)       # global subchunk index of section start
    sub_hi = np.zeros(B, np.int64)
    for g in cfg.groups:
        for i, b in enumerate(g["blks"]):
            lo_s, nlo, hi_s, nhi = g["sec"][i]
            lo_base_j[b] = lo_s * P
            hi_base_j[b] = (hi_s - g["ns_lo"]) * P
            lo_call_col[b] = g["lo_col"]
            hi_call_col[b] = g["hi_col"]
            sub_lo[b] = g["sub_off"] + lo_s
            sub_hi[b] = g["sub_off"] + hi_s

    # per-edge positions
    is_hi = hi_f == 1
    j_call = np.where(is_hi, hi_base_j[blk] + rank, lo_base_j[blk] + rank)
    sub_g = np.where(is_hi, sub_hi[blk], sub_lo[blk]) + rank // P
    p_slot = rank % P
    # flat idx position within the whole lo/hi idx stream
    flat_pos = np.where(is_hi, hi_call_col[blk] * 16, lo_call_col[blk] * 16) + j_call
    idx_val = np.where(is_hi, src - SPLIT, src)

    metas = []
    for c in range(N_CORES):
        m = core == c
        ilo_flat = np.zeros(cfg.L_LO * 16, np.int16)
        ihi_flat = np.zeros(cfg.L_HI * 16, np.int16)
        mlo = m & ~is_hi
        mhi = m & is_hi
        ilo_flat[flat_pos[mlo]] = idx_val[mlo].astype(np.int16)
        ihi_flat[flat_pos[mhi]] = idx_val[mhi].astype(np.int16)
        # wrap j -> [p = j%16, l = j//16], replicate to 128 partitions
        ilo_w = np.tile(ilo_flat.reshape(-1, 16).T, (8, 1))
        ihi_w = np.tile(ihi_flat.reshape(-1, 16).T, (8, 1))

        # fp8 one-hot matrices, built directly on the host
        # oh[p, s, n] = 1.0  iff edge at slot (p, s) routes to local node n
        # ohT[n, s*128+p] = same, node-partitioned (for er routing)
        rel = (loc[m] % P).astype(np.int64)
        sg = sub_g[m]
        pp = p_slot[m]
        oh_u8 = np.zeros(P * cfg.S * P, np.uint8)
        oh_u8[(pp * cfg.S + sg) * P + rel] = FP8_ONE
        oh_u8 = oh_u8.reshape(P, cfg.S * P)
        ohT_u8 = np.zeros(P * cfg.S * P, np.uint8)
        ohT_u8[(rel * cfg.S + sg) * P + pp] = FP8_ONE
        ohT_u8 = ohT_u8.reshape(P, cfg.S * P)

        node = c * npc + np.arange(B)[None, :] * P + np.arange(P)[:, None]
        node = np.where(node < N, node, 0).astype(np.int32)
        metas.append({
            "ilo": np.ascontiguousarray(ilo_w),
            "ihi": np.ascontiguousarray(ihi_w),
            "oh": oh_u8,
            "ohT": ohT_u8,
            "blknode": np.ascontiguousarray(node),
        })

    featT = np.zeros((d_in, cfg.npad), ml_dtypes.bfloat16)
    featT[:, :N] = feat.T.astype(ml_dtypes.bfloat16)
    return cfg, featT, wcomb, metas


def build_program(cfg: Cfg):
    nc = bacc.Bacc("TRN2", target_bir_lowering=False, debug=False,
                   num_devices=N_CORES, num_swdge_queues=2)

    n_hi_rows = cfg.npad - SPLIT
    featT = nc.dram_tensor("featT", [cfg.d_in, cfg.npad], BF16,
                           kind="ExternalInput").ap()
    wcomb = nc.dram_tensor("wcomb", [cfg.d_in, cfg.hd + 2 * cfg.H], BF16,
                           kind="ExternalInput").ap()
    ilo = nc.dram_tensor("ilo", [P, cfg.L_LO], I16, kind="ExternalInput").ap()
    ihi = nc.dram_tensor("ihi", [P, cfg.L_HI], I16, kind="ExternalInput").ap()
    oh_d = nc.dram_tensor("oh", [P, cfg.S * P], U8, kind="ExternalInput").ap()
    ohT_d = nc.dram_tensor("ohT", [P, cfg.S * P], U8,
                           kind="ExternalInput").ap()
    blknode = nc.dram_tensor("blknode", [P, cfg.B], I32,
                             kind="ExternalInput").ap()
    out = nc.dram_tensor("out", [cfg.n_loc_pad, cfg.hd], F32,
                         kind="ExternalOutput").ap()
    ftel_lo = nc.dram_tensor("ftel_lo", [SPLIT, ROW], BF16).ap()
    ftel_hi = nc.dram_tensor("ftel_hi", [n_hi_rows, ROW], BF16).ap()
    er_tab = nc.dram_tensor("er_tab", [cfg.npad, cfg.H], BF16).ap()

    dbg_phase = int(os.environ.get("DBG_PHASE", "0"))
    with tile.TileContext(nc) as tc, ExitStack() as ctx:
        if dbg_phase in (0, 1):
            _gemm_phase(ctx, tc, cfg, featT, wcomb, ftel_lo, ftel_hi, er_tab)
        if dbg_phase in (0, 2):
            _edge_phase(ctx, tc, cfg, ftel_lo, ftel_hi, er_tab,
                        ilo, ihi, oh_d, ohT_d, blknode, out)
    nc.compile()
    return nc


def _gemm_phase(ctx, tc, cfg, featT, wcomb, ftel_lo, ftel_hi, er_tab):
    nc = tc.nc
    GT = 16
    ntiles = cfg.npad // P
    wid = cfg.hd + 2 * cfg.H        # 264
    fe = cfg.hd + cfg.H             # 260: [ft | el]

    wpool = ctx.enter_context(tc.tile_pool(name="wcomb", bufs=1))
    lpool = ctx.enter_context(tc.tile_pool(name="featT_stage", bufs=3))
    spool = ctx.enter_context(tc.tile_pool(name="ftel_stage", bufs=3))
    pspool = ctx.enter_context(tc.tile_pool(name="gemm_ps", bufs=3,
                                            space="PSUM"))

    w_sb = wpool.tile([P, cfg.KH, wid], BF16)
    for k in range(cfg.KH):
        nc.sync.dma_start(out=w_sb[:, k, :], in_=wcomb[k * P:(k + 1) * P, :])

    for g0 in range(0, ntiles, GT):
        gt = min(GT, ntiles - g0)
        c0 = g0 * P
        cols = gt * P
        stage_in = lpool.tile([P, cfg.KH, GT * P], BF16, tag="featT_stage")
        for k in range(cfg.KH):
            nc.sync.dma_start(out=stage_in[:, k, 0:cols],
                              in_=featT[k * P:(k + 1) * P, c0:c0 + cols])
        stage_out = spool.tile([P, GT, wid], BF16, tag="ftel_stage")
        for t in range(gt):
            ps = pspool.tile([P, wid], F32)
            for k in range(cfg.KH):
                nc.tensor.matmul(out=ps[:, :],
                                 lhsT=stage_in[:, k, t * P:(t + 1) * P],
                                 rhs=w_sb[:, k, :],
                                 start=(k == 0), stop=(k == cfg.KH - 1))
            if t % 2 == 0:
                nc.scalar.copy(out=stage_out[:, t, :], in_=ps[:, :])
            else:
                nc.vector.tensor_copy(out=stage_out[:, t, :], in_=ps[:, :])
        if c0 >= SPLIT:
            dst = ftel_hi[c0 - SPLIT:c0 - SPLIT + cols, 0:fe]
        else:
            assert c0 + cols <= SPLIT
            dst = ftel_lo[c0:c0 + cols, 0:fe]
        nc.sync.dma_start(out=dst.rearrange("(t p) c -> p t c", p=P),
                          in_=stage_out[:, 0:gt, 0:fe])
        nc.sync.dma_start(
            out=er_tab[c0:c0 + cols, :].rearrange("(t p) c -> p t c", p=P),
            in_=stage_out[:, 0:gt, fe:wid])


def _edge_phase(ctx, tc, cfg, ftel_lo, ftel_hi, er_tab,
                ilo, ihi, oh_d, ohT_d, blknode, out):
    nc = tc.nc
    H, hd = cfg.H, cfg.hd
    fe = hd + H                      # 260
    NSG = cfg.NSG_MAX

    mpool = ctx.enter_context(tc.tile_pool(name="meta", bufs=1))
    gpool = ctx.enter_context(tc.tile_pool(name="gather", bufs=4))
    opool = ctx.enter_context(tc.tile_pool(name="onehot", bufs=3))
    otpool = ctx.enter_context(tc.tile_pool(name="onehotT", bufs=3))
    ipool = ctx.enter_context(tc.tile_pool(name="idx", bufs=4))
    spool = ctx.enter_context(tc.tile_pool(name="score", bufs=2))
    dpool = ctx.enter_context(tc.tile_pool(name="denom", bufs=2))
    outpool = ctx.enter_context(tc.tile_pool(name="outsb", bufs=2))
    aggps = ctx.enter_context(tc.tile_pool(name="agg_ps", bufs=2,
                                           space="PSUM"))
    erps = ctx.enter_context(tc.tile_pool(name="er_ps", bufs=2, space="PSUM"))

    blkn_sb = mpool.tile([P, cfg.B], I32, tag="blkn")
    nc.sync.dma_start(out=blkn_sb[:, :], in_=blknode[:, :])
    LOMAX = max(g["ns_lo"] for g in cfg.groups) * 8
    HIMAX = max(g["ns_hi"] for g in cfg.groups) * 8

    er_all = mpool.tile([P, cfg.B, H], BF16, tag="er_all")

    def fetch_er(b):
        nc.gpsimd.indirect_dma_start(
            out=er_all[:, b, :], out_offset=None, in_=er_tab[:, :],
            in_offset=IndirectOffsetOnAxis(ap=blkn_sb[:, b:b + 1], axis=0))

    dbg_groups = int(os.environ.get("DBG_GROUPS", "0"))
    groups = cfg.groups[:dbg_groups] if dbg_groups else cfg.groups
    for b in range(cfg.B):
        fetch_er(b)
    for gi, g in enumerate(groups):
        ns = g["ns"]
        ns_lo = g["ns_lo"]
        ns_hi = g["ns_hi"]
        sub0 = g["sub_off"]

        gt = gpool.tile([P, NSG, ROW], BF16, tag="g")
        ilo_g = ipool.tile([P, LOMAX], I16, tag="ilo")
        ihi_g = ipool.tile([P, HIMAX], I16, tag="ihi")
        if ns_lo:
            nc.sync.dma_start(
                out=ilo_g[:, 0:ns_lo * 8],
                in_=ilo[:, g["lo_col"]:g["lo_col"] + ns_lo * 8])
            nc.gpsimd.dma_gather(
                out_ap=gt[:, 0:ns_lo, :], in_ap=ftel_lo[:, :],
                idxs_ap=ilo_g[:, 0:ns_lo * 8],
                num_idxs=ns_lo * P, num_idxs_reg=ns_lo * P, elem_size=ROW,
                single_packet=False, queue_num=0)
        if ns_hi:
            nc.sync.dma_start(
                out=ihi_g[:, 0:ns_hi * 8],
                in_=ihi[:, g["hi_col"]:g["hi_col"] + ns_hi * 8])
            nc.gpsimd.dma_gather(
                out_ap=gt[:, ns_lo:ns, :], in_ap=ftel_hi[:, :],
                idxs_ap=ihi_g[:, 0:ns_hi * 8],
                num_idxs=ns_hi * P, num_idxs_reg=ns_hi * P, elem_size=ROW,
                single_packet=False, queue_num=1)

        oh = opool.tile([P, NSG * P], U8, tag="oh")
        nc.sync.dma_start(out=oh[:, 0:ns * P],
                          in_=oh_d[:, sub0 * P:(sub0 + ns) * P])
        ohT = otpool.tile([P, NSG * P], U8, tag="ohT")
        nc.sync.dma_start(out=ohT[:, 0:ns * P],
                          in_=ohT_d[:, sub0 * P:(sub0 + ns) * P])

        ps_er = erps.tile([P, NSG * H], F32)
        for i, b in enumerate(g["blks"]):
            lo_s, nlo, hi_s, nhi = g["sec"][i]
            for s in list(range(lo_s, lo_s + nlo)) + \
                     list(range(hi_s, hi_s + nhi)):
                nc.tensor.matmul(out=ps_er[:, s * H:(s + 1) * H],
                                 lhsT=ohT[:, s * P:(s + 1) * P].bitcast(FP8),
                                 rhs=er_all[:, b, :], start=True, stop=True)

        sc = spool.tile([P, NSG, H], F32, tag="sc")
        nc.vector.tensor_tensor(
            out=sc[:, 0:ns, :], in0=gt[:, 0:ns, hd:fe],
            in1=ps_er[:, 0:ns * H].rearrange("p (s h) -> p s h", h=H),
            op=OP.add)
        lk = spool.tile([P, NSG, H], F32, tag="lk")
        nc.vector.scalar_tensor_tensor(out=lk[:, 0:ns, :], in0=sc[:, 0:ns, :],
                                       scalar=NEG_SLOPE, in1=sc[:, 0:ns, :],
                                       op0=OP.mult, op1=OP.max)
        w = spool.tile([P, NSG, H], BF16, tag="w")
        nc.scalar.activation(out=w[:, 0:ns, :], in_=lk[:, 0:ns, :], func=AF.Exp)

        # build rhs = [w*ft | w] in place in the gathered tile: the ft cols
        # are scaled elementwise (1:1 streaming, no hazard) and w overwrites
        # the el cols, which the score op has already consumed
        rhs = gt
        nc.vector.tensor_tensor(
            out=rhs[:, 0:ns, 0:hd].rearrange("p s (h d) -> p s h d", h=H),
            in0=gt[:, 0:ns, 0:hd].rearrange("p s (h d) -> p s h d", h=H),
            in1=w[:, 0:ns, :].to_broadcast([P, ns, H, cfg.D]),
            op=OP.mult)
        nc.scalar.copy(out=rhs[:, 0:ns, hd:fe], in_=w[:, 0:ns, :])

        for i, b in enumerate(g["blks"]):
            lo_s, nlo, hi_s, nhi = g["sec"][i]
            subs = list(range(lo_s, lo_s + nlo)) + \
                   list(range(hi_s, hi_s + nhi))
            ps = aggps.tile([P, fe], F32, tag="agg")
            if not subs:
                outsb = outpool.tile([P, hd], F32, tag="out")
                nc.vector.memset(outsb[:, :], 0.0)
                nc.sync.dma_start(out=out[b * P:(b + 1) * P, :],
                                  in_=outsb[:, :])
                continue
            for j, s in enumerate(subs):
                nc.tensor.matmul(out=ps[:, :],
                                 lhsT=oh[:, s * P:(s + 1) * P].bitcast(FP8),
                                 rhs=rhs[:, s, 0:fe],
                                 start=(j == 0), stop=(j == len(subs) - 1))
            den = dpool.tile([P, H], F32, tag="den")
            nc.vector.tensor_scalar_add(out=den[:, :], in0=ps[:, hd:fe],
                                        scalar1=1e-30)
            recip = dpool.tile([P, H], F32, tag="recip")
            nc.vector.reciprocal(out=recip[:, :], in_=den[:, :])
            outsb = outpool.tile([P, hd], F32, tag="out")
            nc.vector.tensor_tensor(
                out=outsb[:, :].rearrange("p (h d) -> p h d", h=H),
                in0=ps[:, 0:hd].rearrange("p (h d) -> p h d", h=H),
                in1=recip[:, :].to_broadcast([P, H, cfg.D]),
                op=OP.mult)
            nc.sync.dma_start(out=out[b * P:(b + 1) * P, :], in_=outsb[:, :])


def kernel(feat, src, dst, W, attn_l, attn_r):
    global LAST_RESULTS
    cfg, featT, wcomb, metas = host_prep(feat, src, dst, W, attn_l, attn_r)

    nc = _PROGRAM_CACHE.get(cfg.key())
    if nc is None:
        nc = build_program(cfg)
        _PROGRAM_CACHE[cfg.key()] = nc

    in_maps = []
    for c in range(N_CORES):
        m = {"featT": featT, "wcomb": wcomb}
        m.update(metas[c])
        in_maps.append(m)

    dbg_cores = int(os.environ.get("DBG_CORES", str(N_CORES)))
    res = run_bass_kernel_spmd(nc, in_maps[:dbg_cores],
                               list(range(dbg_cores)))
    LAST_RESULTS = res

    N, hd = cfg.N, cfg.hd
    out_full = np.zeros((N, hd), np.float32)
    for c in range(dbg_cores):
        lo = c * cfg.npc
        hi = min(lo + cfg.npc, N)
        if hi > lo:
            out_full[lo:hi] = res.results[c]["out"][:hi - lo]
    return out_full.reshape(N, cfg.H, cfg.D)
